# revision 2
# baseline (speedup 1.0000x reference)
"""Trainium2 Bass kernel for CpuLstmModel (LSTM over basins).

Reference computation (per timestep t):
    x0    = relu(x_t @ W_in.T + b_in)                    # [G, H]
    gates = x0 @ w_ih.T + b_ih + h @ w_hh.T + b_hh       # [G, 4H]
    i,f,g,o = split(gates, 4)
    c = sigmoid(f)*c + sigmoid(i)*tanh(g)
    h = sigmoid(o)*tanh(c)
    y_t = h @ W_out.T + b_out                            # [G, 1]

Sharding: data-parallel over ngrid (4096 basins) across 8 cores; weights and
h/c state replicated. On-chip layout is feature-major: activations live as
[hid, basins] tiles, basins are the N=512 moving dim.

Perf structure (vs the plain fp32r version):
  * The recurrent matmul h @ w_hh.T runs in fp8 (e4m3) with
    MatmulPerfMode.DoubleRow: one instruction contracts 2 k-slices of 128 at
    0.5 cycles/row, halving (or better) its tensor-engine time. h and w_hh
    quantization adds ~7e-3 rel err (measured in simulation) vs the 2e-2
    budget; the precision-critical input-side matmul x0 @ w_ih.T stays fp32r.
    Both gate weight matrices are pre-scaled by 32 so fp8 weights stay in the
    e4m3 normal range; the gate activations descale by 1/32.
  * Software pipelining: step t computes linearIn for t+1 and the output
    matmul for t-1, so the tensor engine has ~28 independent matmuls queued
    when it reaches the first h-dependent matmul of a step (covers the
    elementwise tail of the previous step).
  * Gate-group x-side matmuls are hoisted ~5 groups ahead of their h-side
    matmuls (PSUM accumulation groups stay open in between).
"""

import numpy as np
import ml_dtypes

import concourse.bass as bass
import concourse.mybir as mybir
import concourse.tile as tile
from concourse import bacc
from concourse.bass import ds, ts
from concourse.bass_utils import run_bass_kernel_spmd

NT = 365
NGRID = 4096
NX = 32
HID = 512
NY = 1
N_CORES = 8
G = NGRID // N_CORES  # basins per core = 512
KC = HID // 128  # hid chunks = 4
NGATE = 4 * HID  # 2048
MC = NGATE // 128  # gate chunks = 16
WS = 32.0  # gate-weight prescale, descaled in the gate activations

F32 = mybir.dt.float32
F32R = mybir.dt.float32r
F8 = mybir.dt.float8e4
AF = mybir.ActivationFunctionType
DR = mybir.MatmulPerfMode.DoubleRow

U = 28  # steps per hardware-loop iteration; EVEN and divides NT-1=364

gate_funcs = [AF.Sigmoid, AF.Sigmoid, AF.Tanh, AF.Sigmoid]  # i, f, g, o


def build_program(nt=NT, unroll=U, use_loop=True, hoist=5):
    nc = bacc.Bacc("TRN2", num_devices=N_CORES)

    xt_d = nc.dram_tensor("xt", [(nt + 2) * NX, G], F32, kind="ExternalInput").ap()
    wih_d = nc.dram_tensor("wihT", [HID, NGATE], F32, kind="ExternalInput").ap()
    whh8_d = nc.dram_tensor("whh8", [128, KC, NGATE], F8, kind="ExternalInput").ap()
    win_d = nc.dram_tensor("winT", [NX, HID], F32, kind="ExternalInput").ap()
    wout_d = nc.dram_tensor("woutC", [128, KC], F32, kind="ExternalInput").ap()
    bin_d = nc.dram_tensor("binC", [128, KC], F32, kind="ExternalInput").ap()
    bg_d = nc.dram_tensor("bgC", [128, MC], F32, kind="ExternalInput").ap()
    bout_d = nc.dram_tensor("bout", [1, 1], F32, kind="ExternalInput").ap()
    y_d = nc.dram_tensor("y", [nt, G], F32, kind="ExternalOutput").ap()

    with tile.TileContext(nc) as tc:
        with (
            tc.tile_pool(name="const", bufs=1) as cpool,
            tc.tile_pool(name="stag", bufs=2) as stag_pool,
            tc.tile_pool(name="acts", bufs=10) as act_pool,
            tc.tile_pool(name="tmp", bufs=6) as tmp_pool,
            tc.tile_pool(name="state", bufs=1) as state_pool,
            tc.tile_pool(name="ysb", bufs=2) as y_pool,
            tc.tile_pool(name="gpsum", bufs=6, space="PSUM") as gpsum,
            tc.tile_pool(name="xpsum", bufs=1, space="PSUM") as xpsum,
            tc.tile_pool(name="ypsum", bufs=1, space="PSUM") as ypsum,
        ):
            # ---- load weights ----
            w_ih_r = cpool.tile([128, KC * NGATE], F32R, name="w_ih_r", tag="w_ih_r")
            for k in range(KC):
                st = stag_pool.tile([128, NGATE], F32, name="st", tag="st")
                nc.gpsimd.dma_start(st[:], wih_d[ts(k, 128), :])
                nc.vector.tensor_copy(w_ih_r[:, ts(k, NGATE)], st[:])
            w_hh8 = cpool.tile([128, KC, NGATE], F8, name="w_hh8", tag="w_hh8")
            nc.gpsimd.dma_start(w_hh8[:], whh8_d[:, :, :])
            stw = stag_pool.tile([NX, HID], F32, name="stw", tag="stw")
            nc.gpsimd.dma_start(stw[:], win_d[:, :])
            w_in_r = cpool.tile([NX, HID], F32R, name="w_in_r", tag="w_in_r")
            nc.vector.tensor_copy(w_in_r[:], stw[:])
            sto = stag_pool.tile([128, KC], F32, name="sto", tag="sto")
            nc.gpsimd.dma_start(sto[:], wout_d[:, :])
            w_out_r = cpool.tile([128, KC], F32R, name="w_out_r", tag="w_out_r")
            nc.vector.tensor_copy(w_out_r[:], sto[:])
            b_in_sb = cpool.tile([128, KC], F32, name="b_in_sb", tag="b_in_sb")
            nc.gpsimd.dma_start(b_in_sb[:], bin_d[:, :])
            b_g_sb = cpool.tile([128, MC], F32, name="b_g_sb", tag="b_g_sb")
            nc.gpsimd.dma_start(b_g_sb[:], bg_d[:, :])
            b_out_sb = cpool.tile([1, 1], F32, name="b_out_sb", tag="b_out_sb")
            nc.gpsimd.dma_start(b_out_sb[:], bout_d[:, :])

            # ---- persistent state, all ping-pong across step parity ----
            h_f = [
                [state_pool.tile([128, G], F32R, name=f"h{p}{j}", tag=f"h{p}{j}") for j in range(KC)]
                for p in range(2)
            ]
            h8 = [
                state_pool.tile([128, KC, G], F8, name=f"h8{p}", tag=f"h8{p}")
                for p in range(2)
            ]
            c_t = [state_pool.tile([128, G], F32, name=f"c{j}", tag=f"c{j}") for j in range(KC)]
            xt_sb = [
                state_pool.tile([NX, G], F32, name=f"xts{p}", tag=f"xts{p}") for p in range(2)
            ]
            xt_r = [
                state_pool.tile([NX, G], F32R, name=f"xtr{p}", tag=f"xtr{p}") for p in range(2)
            ]
            x0_r = [
                [state_pool.tile([128, G], F32R, name=f"x0{p}{m}", tag=f"x0{p}{m}") for m in range(KC)]
                for p in range(2)
            ]

            def prefetch(t):  # t may be symbolic; parity must be literal
                nc.gpsimd.dma_start(xt_sb[t[1] % 2][:], xt_d[ts(t[0], NX), :])

            def linear_in(t, pe_filler):
                """Emit linearIn for step t; pe_filler: list of thunks emitting
                independent PE work, popped between chunks to cover the
                single-bank xpsum WAR latency."""
                p = t[1] % 2
                nc.vector.tensor_copy(xt_r[p][:], xt_sb[p][:])
                for m in range(KC):
                    xps = xpsum.tile([128, G], F32, name="xps", tag="xps")
                    nc.tensor.matmul(
                        xps[:], w_in_r[:, ts(m, 128)], xt_r[p][:], start=True, stop=True
                    )
                    nc.scalar.activation(
                        x0_r[p][m][:], xps[:], AF.Relu, bias=b_in_sb[:, m : m + 1]
                    )
                    if pe_filler:
                        pe_filler.pop(0)()

            def y_out(t):  # t = (addr, parity) of the step whose h it reads
                p = t[1] % 2
                yps = ypsum.tile([1, G], F32, name="yps", tag="yps")
                for k in range(KC):
                    nc.tensor.matmul(
                        yps[:],
                        w_out_r[:, k : k + 1],
                        h_f[p][k][:],
                        start=(k == 0),
                        stop=(k == KC - 1),
                    )
                y_sb = y_pool.tile([1, G], F32, name="y_sb", tag="y_sb")
                nc.scalar.activation(y_sb[:], yps[:], AF.Identity, bias=b_out_sb[:, 0:1])
                nc.gpsimd.dma_start(y_d[ds(t[0], 1)], y_sb[:])

            def step(t_sym, parity, first, emit_y, emit_prefetch=True):
                """One LSTM step. t_sym: symbolic/int step index, parity: t%2."""
                pp = (parity + 1) % 2  # parity of t-1 / t+1
                hp, hc = h_f[pp], h_f[parity]
                h8p, h8c = h8[pp], h8[parity]

                if emit_prefetch:
                    prefetch((t_sym + 2, parity))

                seq = [(gi * KC + j, j, gi) for j in range(KC) for gi in range(4)]
                groups = {}  # m -> psum tile with x-side accumulated

                def x_phase(idx):
                    m, j, gi = seq[idx]
                    gps = gpsum.tile([128, G], F32, name="gps", tag="gps")
                    for k in range(KC):
                        nc.tensor.matmul(
                            gps[:],
                            w_ih_r[:, ds(k * NGATE + m * 128, 128)],
                            x0_r[parity][k][:],
                            start=(k == 0),
                            stop=(first and k == KC - 1),
                        )
                    groups[idx] = gps

                def h_phase(idx):
                    m, j, gi = seq[idx]
                    gps = groups.pop(idx)
                    if not first:
                        for p8 in range(2):
                            nc.tensor.matmul(
                                gps[:],
                                w_hh8[:, 2 * p8 : 2 * p8 + 2, ts(m, 128)],
                                h8p[:, 2 * p8 : 2 * p8 + 2, :],
                                start=False,
                                stop=(p8 == 1),
                                perf_mode=DR,
                            )
                    a = act_pool.tile([128, G], F32, name="act", tag="act")
                    nc.scalar.activation(
                        a[:], gps[:], gate_funcs[gi], bias=b_g_sb[:, m : m + 1], scale=1.0 / WS
                    )
                    return a

                # linearIn for t+1 interleaved with y(t-1) and early x-phases
                filler = []
                if emit_y:
                    filler.append(lambda: y_out((t_sym - 1, pp)))
                nh = min(hoist, len(seq))
                for i in range(nh):
                    filler.append(lambda i=i: x_phase(i))
                linear_in((t_sym + 1, pp), filler)
                for f in filler:  # anything not consumed as filler
                    f()

                acts = []
                for idx in range(len(seq)):
                    if idx + nh < len(seq):
                        x_phase(idx + nh)
                    m, j, gi = seq[idx]
                    acts.append(h_phase(idx))
                    if gi == 3:  # all four gates of hid-chunk j done
                        a_i, a_f, a_g, a_o = acts
                        acts = []
                        if first:
                            nc.vector.tensor_mul(c_t[j][:], a_i[:], a_g[:])
                        else:
                            t1 = tmp_pool.tile([128, G], F32, name="t1", tag="t1")
                            nc.vector.tensor_mul(t1[:], a_f[:], c_t[j][:])
                            t2 = tmp_pool.tile([128, G], F32, name="t2", tag="t2")
                            nc.vector.tensor_mul(t2[:], a_i[:], a_g[:])
                            nc.vector.tensor_add(c_t[j][:], t1[:], t2[:])
                        tanc = tmp_pool.tile([128, G], F32, name="tanc", tag="tanc")
                        nc.scalar.activation(tanc[:], c_t[j][:], AF.Tanh)
                        nc.vector.tensor_mul(hc[j][:], a_o[:], tanc[:])
                        nc.gpsimd.tensor_copy(h8c[:, j, :], hc[j][:])

            # ---- prologue: xt(0), xt(1), linearIn(0) ----
            prefetch((0, 0))
            prefetch((1, 1))
            linear_in((0, 0), [])

            # ---- step 0 (no h recurrence, no y yet) ----
            step(0, 0, first=True, emit_y=False)

            # ---- steps 1..nt-1 ----
            if use_loop:
                assert (nt - 1) % unroll == 0 and unroll % 2 == 0
                with tc.For_i(1, nt, unroll) as iv:
                    for u in range(unroll):
                        step(iv + u, (1 + u) % 2, first=False, emit_y=True)
            else:
                for t in range(1, nt):
                    step(t, t % 2, first=False, emit_y=True)

            # ---- epilogue: y(nt-1) ----
            y_out((nt - 1, (nt - 1) % 2))

    nc.compile()
    return nc


def _prep_inputs(nt, inputs, W_in, b_in, w_ih, w_hh, b_ih, b_hh, W_out, b_out):
    f = np.float32
    inputs = np.ascontiguousarray(np.asarray(inputs, f))
    wihT = np.ascontiguousarray((WS * np.asarray(w_ih, f)).T)  # [HID, 4H]
    whh = (WS * np.asarray(w_hh, f)).T  # [HID, 4H]
    whh8 = np.ascontiguousarray(
        whh.reshape(KC, 128, NGATE).transpose(1, 0, 2)
    ).astype(ml_dtypes.float8_e4m3)  # [k(128), s(KC), m(4H)]
    winT = np.ascontiguousarray(np.asarray(W_in, f).T)  # [NX, HID]
    woutC = np.ascontiguousarray(np.asarray(W_out, f).reshape(NY, KC, 128)[0].T)
    binC = np.ascontiguousarray(np.asarray(b_in, f).reshape(KC, 128).T)
    bgC = np.ascontiguousarray(
        (np.asarray(b_ih, f) + np.asarray(b_hh, f)).reshape(MC, 128).T
    )
    bout = np.asarray(b_out, f).reshape(1, 1)
    shared = dict(
        wihT=wihT, whh8=whh8, winT=winT, woutC=woutC, binC=binC, bgC=bgC, bout=bout
    )
    in_maps = []
    for c in range(N_CORES):
        xc = inputs[:nt, c * G : (c + 1) * G, :]  # [nt, G, NX]
        xt = np.ascontiguousarray(xc.transpose(0, 2, 1)).reshape(nt * NX, G)
        xt_pad = np.zeros(((nt + 2) * NX, G), f)
        xt_pad[: nt * NX] = xt
        in_maps.append({"xt": xt_pad, **shared})
    return in_maps


def run(inputs_dict, trace=False, nt=NT, unroll=U, use_loop=True, **spmd_kwargs):
    nc = build_program(nt, unroll, use_loop)
    in_maps = _prep_inputs(nt, **inputs_dict)
    res = run_bass_kernel_spmd(
        nc, in_maps, core_ids=list(range(N_CORES)), trace=trace, **spmd_kwargs
    )
    out = np.empty((nt, NGRID, NY), np.float32)
    for c in range(N_CORES):
        out[:, c * G : (c + 1) * G, 0] = res.results[c]["y"]
    return out, res


def kernel(**inputs):
    out, _ = run(inputs, trace=False)
    return out
